# revision 11
# baseline (speedup 1.0000x reference)
"""DDNLoss (depth distribution network focal loss) on 8 trn2 NeuronCores.

Data-parallel over B (1 image per core, B=8). v5: original (channel-
partition) layout, PE-reduce pipeline with the v4 schedule stalls fixed:

  * Rasterize in the BIN domain: host converts per-box depths to exact
    f32 LID bin indices (monotone, so min over boxes commutes with
    binning); raster is min/max of small ints, exact in bf16. Four
    interleaved DVE chains (ts 4x cand + TT 2x min-fold) shorten the
    serial head to ~6us; column masks DMA-broadcast across 4 queues.
  * Select mask via tensor_scalar is_equal (4x DVE mode) + TT mult (2x).
  * S and a reduce matmuls in SEPARATE quadrant-packed PSUM tiles
    (4 x 468-col blocks per 1-bank tile at bases 0/32/64/96); the
    a-stream lags the S-stream by one chunk so PE starts on S as soon
    as the first exp lands (~8us) and never waits for the raster/t path.
  * Drains mostly on ACT (DVE is loaded with raster+eq+am), one batched
    DMA per stage tile on the gpsimd/sync queues.
  * t broadcast rows prefetched for all chunks right after the bounce.

Host sums the 8x128 partials -> scalar loss.
"""

import numpy as np
from contextlib import ExitStack

import concourse.bass as bass
import concourse.bacc as bacc_mod
import concourse.tile as tile
import concourse.mybir as mybir
from concourse.bass_utils import run_bass_kernel_spmd

try:
    import ml_dtypes
    _BF16 = ml_dtypes.bfloat16
except Exception:  # pragma: no cover
    _BF16 = None

# Problem constants (hardcoded per contract)
B, C, H, W, N = 8, 81, 96, 312, 32
HW = H * W                      # 29952
NCH = 8                         # logits chunks
CH = HW // NCH                  # 3744
QB = 468                        # matmul column block (<= 512 psum bank)
TPX = 4 * QB                    # pixels per PSUM tile (1872), 2 tiles/chunk

ALPHA = 0.25
D_MIN, D_MAX, NUM_BINS = 0.001, 60.0, 80
BIN_SIZE = 2.0 * (D_MAX - D_MIN) / (NUM_BINS * (1 + NUM_BINS))
BIGBIN = 128.0                  # empty marker; exact in bf16, > any bin
C0 = -ALPHA / float(B * HW)     # fold -alpha and global pixel normalizer

LAST_RESULTS = None


def build_program():
    f32 = mybir.dt.float32
    bf16 = mybir.dt.bfloat16
    Alu = mybir.AluOpType
    Act = mybir.ActivationFunctionType

    nc = bacc_mod.Bacc("TRN2", target_bir_lowering=False)
    logits = nc.dram_tensor("logits", [C, HW], bf16, kind="ExternalInput")
    rowpen = nc.dram_tensor("rowpen", [H, N], f32, kind="ExternalInput")
    colbin = nc.dram_tensor("colbin", [N, W], bf16, kind="ExternalInput")
    iota81 = nc.dram_tensor("iota81", [C, 1], f32, kind="ExternalInput")
    ones81 = nc.dram_tensor("ones81", [C, 1], bf16, kind="ExternalInput")
    partial = nc.dram_tensor("partial", [128, 1], f32, kind="ExternalOutput")
    tprobe = nc.dram_tensor("tprobe", [1, HW], bf16, kind="ExternalOutput")

    with ExitStack() as ctx:
        tc = ctx.enter_context(tile.TileContext(nc))
        consts = ctx.enter_context(tc.tile_pool(name="consts", bufs=1))
        rast = ctx.enter_context(tc.tile_pool(name="rast", bufs=1))
        lg = ctx.enter_context(tc.tile_pool(name="lg", bufs=3))
        tb_pool = ctx.enter_context(tc.tile_pool(name="tb", bufs=1))
        ex = ctx.enter_context(tc.tile_pool(name="ex", bufs=2))
        eqp = ctx.enter_context(tc.tile_pool(name="eq", bufs=2))
        amp = ctx.enter_context(tc.tile_pool(name="am", bufs=2))
        st_pool = ctx.enter_context(tc.tile_pool(name="st", bufs=4))
        fin = ctx.enter_context(tc.tile_pool(name="fin", bufs=1))
        psu = ctx.enter_context(tc.tile_pool(name="psu", bufs=8, space="PSUM"))
        dr = ctx.enter_context(tc.tile_pool(name="dr", bufs=1, space="DRAM"))

        # ---- constants
        c_iota81 = consts.tile([C, 1], f32)
        nc.sync.dma_start(c_iota81[:], iota81[:, :])
        c_ones81 = consts.tile([C, 1], bf16)
        nc.sync.dma_start(c_ones81[:], ones81[:, :])
        c_rowpen = consts.tile([H, N], f32)
        nc.sync.dma_start(c_rowpen[:], rowpen[:, :])

        # first logits chunk as early as possible (gates the first exp/S)
        L_tiles = [None] * NCH
        L0 = lg.tile([C, CH], bf16, tag="L")
        L_tiles[0] = L0
        nc.sync.dma_start(L_tiles[0][:], logits[:, 0:CH])

        # column-mask rows broadcast to all H partitions (stride-0 DMA),
        # spread across sync / gpsimd / tensor / scalar queues
        bcast_q = [nc.sync, nc.gpsimd, nc.scalar, nc.gpsimd]
        c_cb = []
        for n in range(N):
            cbn = rast.tile([H, W], bf16, tag=f"cb{n}")
            bcast_q[n % 4].dma_start(cbn[:], colbin[n:n + 1, :].broadcast_to((H, W)))
            c_cb.append(cbn)

        # ---- raster: T(h,w) = min_n max(rowpen(h,n), colbin(n,w))
        # 4 interleaved DVE chains; per box: ts (4x) + TT min (2x)
        chains = []
        for ci in range(4):
            dm = rast.tile([H, W], bf16, tag=f"dm{ci}")
            nc.vector.memset(dm[:], BIGBIN)
            chains.append(dm)
        cand = [None] * 4
        for n in range(N):
            ci = n % 4
            cn = rast.tile([H, W], bf16, tag=f"cand{ci}", bufs=2)
            nc.vector.tensor_scalar(out=cn[:], in0=c_cb[n][:],
                                    scalar1=c_rowpen[:, n:n + 1], scalar2=None,
                                    op0=Alu.max)
            nc.vector.tensor_tensor(out=chains[ci][:], in0=cn[:],
                                    in1=chains[ci][:], op=Alu.min)
        m01 = rast.tile([H, W], bf16)
        nc.vector.tensor_tensor(out=m01[:], in0=chains[0][:], in1=chains[1][:],
                                op=Alu.min)
        m23 = rast.tile([H, W], bf16)
        nc.vector.tensor_tensor(out=m23[:], in0=chains[2][:], in1=chains[3][:],
                                op=Alu.min)
        T = rast.tile([H, W], bf16)
        nc.vector.tensor_tensor(out=T[:], in0=m01[:], in1=m23[:], op=Alu.min)

        # t = min(T, 80); fg = T < 100; w = 12*fg + 1   (all exact in bf16)
        tt = rast.tile([H, W], bf16)
        nc.vector.tensor_scalar(out=tt[:], in0=T[:], scalar1=80.0,
                                scalar2=None, op0=Alu.min)
        fg = rast.tile([H, W], bf16)
        nc.vector.tensor_scalar(out=fg[:], in0=T[:], scalar1=100.0,
                                scalar2=None, op0=Alu.is_lt)
        wgt = rast.tile([H, W], bf16)
        nc.vector.tensor_scalar(out=wgt[:], in0=fg[:], scalar1=12.0,
                                scalar2=1.0, op0=Alu.mult, op1=Alu.add)
        nc.gpsimd.dma_start(tprobe[0:1, :], tt[:])

        # ---- bounce t and w to DRAM in flat pixel order
        tdram = dr.tile([1, HW], bf16)
        nc.gpsimd.dma_start(tdram[:, :], tt[:])
        wdram = dr.tile([1, HW], bf16)
        nc.gpsimd.dma_start(wdram[:, :], wgt[:])

        # prefetch ALL t-broadcast tiles (they only depend on tdram)
        tb_tiles = []
        for j in range(NCH):
            sl = slice(j * CH, (j + 1) * CH)
            t_b = tb_pool.tile([C, CH], bf16, tag=f"tb{j}")
            q = nc.sync if (j % 2 == 0) else nc.gpsimd
            q.dma_start(t_b[:], tdram[0:1, sl].broadcast_to((C, CH)))
            tb_tiles.append(t_b)

        # S / a rows in DRAM (row 0 = S, row 1 = a), bf16
        sadram = dr.tile([2, HW], bf16)

        # ---- main pipeline.
        # S-matmul stream runs as soon as exp(chunk) is ready; the a-stream
        # (needs t_b -> eq -> am) lags one chunk so PE never stalls on it.
        E_tiles = [None] * NCH
        am_tiles = [None] * NCH

        def emit_exp(j):
            if L_tiles[j] is None:
                sl = slice(j * CH, (j + 1) * CH)
                Lj = lg.tile([C, CH], bf16, tag="L", name=f"L{j}")
                nc.sync.dma_start(Lj[:], logits[:, sl])
                L_tiles[j] = Lj
            E = ex.tile([C, CH], bf16, tag="E", name=f"E{j}")
            nc.scalar.activation(E[:], L_tiles[j][:], Act.Exp)
            E_tiles[j] = E

        def emit_select(j):
            eq = eqp.tile([C, CH], bf16, tag="eq")
            nc.vector.tensor_scalar(out=eq[:], in0=tb_tiles[j][:],
                                    scalar1=c_iota81[:, 0:1], scalar2=None,
                                    op0=Alu.is_equal)
            am = amp.tile([C, CH], bf16, tag="am", name=f"am{j}")
            nc.vector.tensor_tensor(out=am[:], in0=eq[:], in1=L_tiles[j][:],
                                    op=Alu.mult)
            am_tiles[j] = am

        drain_ct = [0]

        def emit_reduce(j, q):
            """q = 0: S from E_tiles[j]; q = 1: a from am_tiles[j]."""
            src = E_tiles[j] if q == 0 else am_tiles[j]
            base = j * CH
            for k in range(0, CH, TPX):
                ps = psu.tile([128, QB], mybir.dt.float32, tag="ps", bufs=8)
                for blk in range(4):
                    o = k + blk * QB
                    nc.tensor.matmul(ps[32 * blk:32 * blk + 1, :],
                                     c_ones81[:, 0:1], src[:, o:o + QB],
                                     start=True, stop=True,
                                     tile_position=(0, 32 * blk))
                stage = st_pool.tile([128, QB], mybir.dt.bfloat16,
                                     tag="stage", name=f"stg{drain_ct[0]}")
                # drains: 3 of 4 on ACT, 1 on DVE
                if drain_ct[0] % 4 == 3:
                    nc.vector.tensor_copy(out=stage[:], in_=ps[:])
                else:
                    nc.scalar.copy(stage[:], ps[:])
                drain_ct[0] += 1
                gb = base + k
                dst = sadram[q:q + 1, gb:gb + TPX].rearrange(
                    "o (b c) -> (o b) c", b=4)
                dq = nc.gpsimd if (drain_ct[0] % 2 == 0) else nc.sync
                dq.dma_start(dst, stage[0:97:32, 0:QB])

        emit_exp(0)
        for j in range(NCH):
            if j + 1 < NCH:
                emit_exp(j + 1)
            emit_reduce(j, 0)        # S stream for chunk j
            emit_select(j)
            if j >= 1:
                emit_reduce(j - 1, 1)  # lagged a stream
        emit_reduce(NCH - 1, 1)

        # ---- reload in (128, 234) slot layout
        NG = HW // 128  # 234
        s_slot = fin.tile([128, NG], bf16)
        nc.sync.dma_start(
            s_slot[:], sadram[0:1, :].rearrange("o (p g) -> (o p) g", p=128))
        a_slot = fin.tile([128, NG], bf16)
        nc.sync.dma_start(
            a_slot[:], sadram[1:2, :].rearrange("o (p g) -> (o p) g", p=128))
        w_slot = fin.tile([128, NG], bf16)
        nc.sync.dma_start(
            w_slot[:], wdram[0:1, :].rearrange("o (p g) -> (o p) g", p=128))

        # ---- focal epilogue on (128, 234)
        # p = exp(a)/S computed while the Exp table is still loaded, so ACT
        # swaps tables only once (Exp -> Ln).
        ea = fin.tile([128, NG], f32)
        nc.scalar.activation(ea[:], a_slot[:], Act.Exp)
        rS = fin.tile([128, NG], f32)
        nc.vector.reciprocal(rS[:], s_slot[:])
        pp = fin.tile([128, NG], f32)
        nc.vector.tensor_tensor(out=pp[:], in0=ea[:], in1=rS[:], op=Alu.mult)
        lnS = fin.tile([128, NG], f32)
        nc.scalar.activation(lnS[:], s_slot[:], Act.Ln)
        logp = fin.tile([128, NG], f32)
        nc.vector.tensor_tensor(out=logp[:], in0=a_slot[:], in1=lnS[:],
                                op=Alu.subtract)
        om = fin.tile([128, NG], f32)
        nc.vector.tensor_scalar(out=om[:], in0=pp[:], scalar1=-1.0,
                                scalar2=1.0, op0=Alu.mult, op1=Alu.add)
        om2 = fin.tile([128, NG], f32)
        nc.vector.tensor_tensor(out=om2[:], in0=om[:], in1=om[:], op=Alu.mult)
        t2 = fin.tile([128, NG], f32)
        nc.vector.scalar_tensor_tensor(
            out=t2[:], in0=om2[:], scalar=C0, in1=logp[:],
            op0=Alu.mult, op1=Alu.mult)
        fs = fin.tile([128, NG], f32)
        acc = fin.tile([128, 1], f32)
        nc.vector.scalar_tensor_tensor(
            out=fs[:], in0=t2[:], scalar=0.0, in1=w_slot[:],
            op0=Alu.add, op1=Alu.mult, accum_out=acc[:])
        nc.sync.dma_start(partial[:, :], acc[:])

    nc.compile()
    return nc


_CACHE = {}


def _get_program():
    if "nc" not in _CACHE:
        _CACHE["nc"] = build_program()
    return _CACHE["nc"]


def _bin_f32(d):
    """Exact f32 replication of the reference LID binning on box depths."""
    d = np.asarray(d, dtype=np.float32)
    idx = np.float32(-0.5) + np.float32(0.5) * np.sqrt(
        np.float32(1.0) + np.float32(8.0) * (d - np.float32(D_MIN))
        / np.float32(BIN_SIZE))
    invalid = (idx < 0) | (idx > NUM_BINS) | ~np.isfinite(idx)
    return np.where(invalid, NUM_BINS, idx.astype(np.int32)).astype(np.float32)


def kernel(depth_logits, gt_boxes2d, num_gt_per_img, gt_center_depth):
    global LAST_RESULTS
    dl = np.ascontiguousarray(np.asarray(depth_logits, dtype=np.float32))
    assert dl.shape == (B, C, H, W), dl.shape
    n_gt = int(num_gt_per_img)
    assert n_gt == N, n_gt
    boxes = np.asarray(gt_boxes2d, dtype=np.float32)
    depth = np.asarray(gt_center_depth, dtype=np.float32)

    u1 = np.floor(boxes[:, 0]).astype(np.int32)
    v1 = np.floor(boxes[:, 1]).astype(np.int32)
    u2 = np.ceil(boxes[:, 2]).astype(np.int32)
    v2 = np.ceil(boxes[:, 3]).astype(np.int32)
    bins = _bin_f32(depth)                                    # (B*N,)
    rows = np.arange(H)[:, None]
    cols = np.arange(W)[None, :]
    iota81 = np.arange(C, dtype=np.float32)[:, None]
    ones81 = np.ones((C, 1), dtype=_BF16)

    logits_flat = dl.reshape(B, C, HW)
    in_maps = []
    for b in range(B):
        sl = slice(b * N, (b + 1) * N)
        bv1, bv2, bu1, bu2 = v1[sl], v2[sl], u1[sl], u2[sl]
        bb = bins[sl]
        rp = np.where((rows >= bv1[None, :]) & (rows < bv2[None, :]),
                      0.0, BIGBIN).astype(np.float32)          # (H, N)
        cb = np.where((cols >= bu1[:, None]) & (cols < bu2[:, None]),
                      bb[:, None], BIGBIN).astype(_BF16)       # (N, W)
        in_maps.append({
            "logits": np.ascontiguousarray(logits_flat[b].astype(_BF16)),
            "rowpen": np.ascontiguousarray(rp),
            "colbin": np.ascontiguousarray(cb),
            "iota81": iota81,
            "ones81": ones81,
        })

    nc = _get_program()
    res = run_bass_kernel_spmd(nc, in_maps, core_ids=list(range(B)))
    LAST_RESULTS = res
    total = np.float64(0.0)
    for r in res.results:
        total += np.asarray(r["partial"], dtype=np.float64).sum()
    return np.float32(total)


if __name__ == "__main__":
    import tempfile
    from concourse.bass_utils import compile_bass_kernel
    compile_bass_kernel(_get_program(), tempfile.mkdtemp())
    print("COMPILE OK")


# revision 13
# speedup vs baseline: 1.0961x; 1.0961x over previous
"""DDNLoss (depth distribution network focal loss) on 8 trn2 NeuronCores.

Data-parallel over B (1 image per core, B=8). v5: original (channel-
partition) layout, PE-reduce pipeline with the v4 schedule stalls fixed:

  * Rasterize in the BIN domain: host converts per-box depths to exact
    f32 LID bin indices (monotone, so min over boxes commutes with
    binning); raster is min/max of small ints, exact in bf16. Four
    interleaved DVE chains (ts 4x cand + TT 2x min-fold) shorten the
    serial head to ~6us; column masks DMA-broadcast across 4 queues.
  * Select mask via tensor_scalar is_equal (4x DVE mode) + TT mult (2x).
  * S and a reduce matmuls in SEPARATE quadrant-packed PSUM tiles
    (4 x 468-col blocks per 1-bank tile at bases 0/32/64/96); the
    a-stream lags the S-stream by one chunk so PE starts on S as soon
    as the first exp lands (~8us) and never waits for the raster/t path.
  * Drains mostly on ACT (DVE is loaded with raster+eq+am), one batched
    DMA per stage tile on the gpsimd/sync queues.
  * t broadcast rows prefetched for all chunks right after the bounce.

Host sums the 8x128 partials -> scalar loss.
"""

import numpy as np
from contextlib import ExitStack

import concourse.bass as bass
import concourse.bacc as bacc_mod
import concourse.tile as tile
import concourse.mybir as mybir
from concourse.bass_utils import run_bass_kernel_spmd

try:
    import ml_dtypes
    _BF16 = ml_dtypes.bfloat16
except Exception:  # pragma: no cover
    _BF16 = None

# Problem constants (hardcoded per contract)
B, C, H, W, N = 8, 81, 96, 312, 32
HW = H * W                      # 29952
NCH = 8                         # logits chunks
CH = HW // NCH                  # 3744
QB = 468                        # matmul column block (<= 512 psum bank)
TPX = 4 * QB                    # pixels per PSUM tile (1872), 2 tiles/chunk

ALPHA = 0.25
D_MIN, D_MAX, NUM_BINS = 0.001, 60.0, 80
BIN_SIZE = 2.0 * (D_MAX - D_MIN) / (NUM_BINS * (1 + NUM_BINS))
BIGBIN = 128.0                  # empty marker; exact in bf16, > any bin
C0 = -ALPHA / float(B * HW)     # fold -alpha and global pixel normalizer

LAST_RESULTS = None


def build_program():
    f32 = mybir.dt.float32
    bf16 = mybir.dt.bfloat16
    Alu = mybir.AluOpType
    Act = mybir.ActivationFunctionType

    nc = bacc_mod.Bacc("TRN2", target_bir_lowering=False)
    logits = nc.dram_tensor("logits", [C, HW], bf16, kind="ExternalInput")
    rowpen = nc.dram_tensor("rowpen", [H, N], f32, kind="ExternalInput")
    colbin = nc.dram_tensor("colbin", [N, W], bf16, kind="ExternalInput")
    iota81 = nc.dram_tensor("iota81", [C, 1], f32, kind="ExternalInput")
    ones81 = nc.dram_tensor("ones81", [C, 1], bf16, kind="ExternalInput")
    partial = nc.dram_tensor("partial", [128, 1], f32, kind="ExternalOutput")
    tprobe = nc.dram_tensor("tprobe", [1, HW], bf16, kind="ExternalOutput")

    with ExitStack() as ctx:
        tc = ctx.enter_context(tile.TileContext(nc))
        consts = ctx.enter_context(tc.tile_pool(name="consts", bufs=1))
        rast = ctx.enter_context(tc.tile_pool(name="rast", bufs=1))
        lg = ctx.enter_context(tc.tile_pool(name="lg", bufs=3))
        tb_pool = ctx.enter_context(tc.tile_pool(name="tb", bufs=1))
        ex = ctx.enter_context(tc.tile_pool(name="ex", bufs=2))
        eqp = ctx.enter_context(tc.tile_pool(name="eq", bufs=2))
        amp = ctx.enter_context(tc.tile_pool(name="am", bufs=2))
        st_pool = ctx.enter_context(tc.tile_pool(name="st", bufs=4))
        fin = ctx.enter_context(tc.tile_pool(name="fin", bufs=1))
        psu = ctx.enter_context(tc.tile_pool(name="psu", bufs=8, space="PSUM"))
        dr = ctx.enter_context(tc.tile_pool(name="dr", bufs=1, space="DRAM"))

        # first logits chunk as early as possible (gates the first exp/S)
        L_tiles = [None] * NCH
        L0 = lg.tile([C, CH], bf16, tag="L")
        L_tiles[0] = L0
        nc.sync.dma_start(L_tiles[0][:], logits[:, 0:CH])

        # ---- constants
        c_iota81 = consts.tile([C, 1], f32)
        nc.sync.dma_start(c_iota81[:], iota81[:, :])
        c_ones81 = consts.tile([C, 1], bf16)
        nc.sync.dma_start(c_ones81[:], ones81[:, :])
        c_rowpen = consts.tile([H, N], f32)
        nc.gpsimd.dma_start(c_rowpen[:], rowpen[:, :])

        # column-mask rows broadcast to all H partitions (stride-0 DMA),
        # spread across sync / gpsimd / tensor / scalar queues
        bcast_q = [nc.sync, nc.gpsimd]
        c_cb = []
        for n in range(N):
            cbn = rast.tile([H, W], bf16, tag=f"cb{n}")
            bcast_q[n % 2].dma_start(cbn[:], colbin[n:n + 1, :].broadcast_to((H, W)))
            c_cb.append(cbn)

        # ---- raster: T(h,w) = min_n max(rowpen(h,n), colbin(n,w))
        # 4 interleaved DVE chains; per box: ts (4x) + TT min (2x)
        chains = []
        for ci in range(4):
            dm = rast.tile([H, W], bf16, tag=f"dm{ci}")
            nc.vector.memset(dm[:], BIGBIN)
            chains.append(dm)
        cand = [None] * 4
        for n in range(N):
            ci = n % 4
            cn = rast.tile([H, W], bf16, tag=f"cand{ci}", bufs=2)
            nc.vector.tensor_scalar(out=cn[:], in0=c_cb[n][:],
                                    scalar1=c_rowpen[:, n:n + 1], scalar2=None,
                                    op0=Alu.max)
            nc.vector.tensor_tensor(out=chains[ci][:], in0=cn[:],
                                    in1=chains[ci][:], op=Alu.min)
        m01 = rast.tile([H, W], bf16)
        nc.vector.tensor_tensor(out=m01[:], in0=chains[0][:], in1=chains[1][:],
                                op=Alu.min)
        m23 = rast.tile([H, W], bf16)
        nc.vector.tensor_tensor(out=m23[:], in0=chains[2][:], in1=chains[3][:],
                                op=Alu.min)
        T = rast.tile([H, W], bf16)
        nc.vector.tensor_tensor(out=T[:], in0=m01[:], in1=m23[:], op=Alu.min)

        # t = min(T, 80); fg = T < 100; w = 12*fg + 1   (all exact in bf16)
        tt = rast.tile([H, W], bf16)
        nc.vector.tensor_scalar(out=tt[:], in0=T[:], scalar1=80.0,
                                scalar2=None, op0=Alu.min)
        fg = rast.tile([H, W], bf16)
        nc.vector.tensor_scalar(out=fg[:], in0=T[:], scalar1=100.0,
                                scalar2=None, op0=Alu.is_lt)
        wgt = rast.tile([H, W], bf16)
        nc.vector.tensor_scalar(out=wgt[:], in0=fg[:], scalar1=12.0,
                                scalar2=1.0, op0=Alu.mult, op1=Alu.add)
        nc.gpsimd.dma_start(tprobe[0:1, :], tt[:])

        # ---- bounce t and w to DRAM in flat pixel order
        tdram = dr.tile([1, HW], bf16)
        nc.gpsimd.dma_start(tdram[:, :], tt[:])
        wdram = dr.tile([1, HW], bf16)
        nc.gpsimd.dma_start(wdram[:, :], wgt[:])

        # prefetch ALL t-broadcast tiles (they only depend on tdram)
        tb_tiles = []
        for j in range(NCH):
            sl = slice(j * CH, (j + 1) * CH)
            t_b = tb_pool.tile([C, CH], bf16, tag=f"tb{j}")
            q = nc.sync if (j % 2 == 0) else nc.gpsimd
            q.dma_start(t_b[:], tdram[0:1, sl].broadcast_to((C, CH)))
            tb_tiles.append(t_b)

        # S / a rows in DRAM (row 0 = S, row 1 = a), bf16
        sadram = dr.tile([2, HW], bf16)

        # ---- main pipeline.
        # S-matmul stream runs as soon as exp(chunk) is ready; the a-stream
        # (needs t_b -> eq -> am) lags one chunk so PE never stalls on it.
        E_tiles = [None] * NCH
        am_tiles = [None] * NCH

        def emit_exp(j):
            if L_tiles[j] is None:
                sl = slice(j * CH, (j + 1) * CH)
                Lj = lg.tile([C, CH], bf16, tag="L", name=f"L{j}")
                nc.sync.dma_start(Lj[:], logits[:, sl])
                L_tiles[j] = Lj
            E = ex.tile([C, CH], bf16, tag="E", name=f"E{j}")
            nc.scalar.activation(E[:], L_tiles[j][:], Act.Exp)
            E_tiles[j] = E

        def emit_select(j):
            eq = eqp.tile([C, CH], bf16, tag="eq")
            nc.vector.tensor_scalar(out=eq[:], in0=tb_tiles[j][:],
                                    scalar1=c_iota81[:, 0:1], scalar2=None,
                                    op0=Alu.is_equal)
            am = amp.tile([C, CH], bf16, tag="am", name=f"am{j}")
            nc.vector.tensor_tensor(out=am[:], in0=eq[:], in1=L_tiles[j][:],
                                    op=Alu.mult)
            am_tiles[j] = am

        drain_ct = [0]

        def emit_reduce(j, q):
            """q = 0: S from E_tiles[j]; q = 1: a from am_tiles[j].

            One 2-bank PSUM tile per (chunk, quantity): 8 matmul outputs at
            (base b in 0/32/64/96) x (column half h), block (b, h) covering
            pixel range h*1872 + b*468. One drain copy + one batched DMA.
            """
            src = E_tiles[j] if q == 0 else am_tiles[j]
            base = j * CH
            # column half h sits at bank-aligned offset h*512 so no matmul
            # output crosses a PSUM bank boundary
            ps = psu.tile([128, 1024], mybir.dt.float32, tag="ps", bufs=4,
                          name=f"ps{j}_{q}")
            for h in range(2):
                for blk in range(4):
                    o = h * 4 * QB + blk * QB
                    nc.tensor.matmul(ps[32 * blk:32 * blk + 1,
                                        h * 512:h * 512 + QB],
                                     c_ones81[:, 0:1], src[:, o:o + QB],
                                     start=True, stop=True,
                                     tile_position=(0, 32 * blk))
            stage = st_pool.tile([128, 1024], mybir.dt.bfloat16,
                                 tag="stage", name=f"stg{drain_ct[0]}")
            # drains: S (and late a) on ACT, early a on DVE -- keeps the ACT
            # queue free of anything that waits on the am/DVE chain while
            # exps are still being produced.
            if q == 1 and j < 4:
                nc.vector.tensor_copy(out=stage[:], in_=ps[:])
            else:
                nc.scalar.copy(stage[:], ps[:])
            drain_ct[0] += 1
            dst = sadram[q:q + 1, base:base + CH].rearrange(
                "o (h b c) -> o b h c", b=4, h=2)
            src_v = stage[0:97:32, :].rearrange(
                "p (h x) -> p h x", h=2)[:, :, 0:QB]
            dq = nc.sync if q == 0 else nc.gpsimd
            dq.dma_start(dst, src_v)

        emit_exp(0)
        emit_exp(1)
        for j in range(NCH):
            if j + 2 < NCH:
                emit_exp(j + 2)
            emit_reduce(j, 0)        # S stream for chunk j
            emit_select(j)
            if j >= 1:
                emit_reduce(j - 1, 1)  # lagged a stream
        emit_reduce(NCH - 1, 1)

        # ---- reload in (128, 234) slot layout
        NG = HW // 128  # 234
        s_slot = fin.tile([128, NG], bf16)
        nc.sync.dma_start(
            s_slot[:], sadram[0:1, :].rearrange("o (p g) -> (o p) g", p=128))
        a_slot = fin.tile([128, NG], bf16)
        nc.sync.dma_start(
            a_slot[:], sadram[1:2, :].rearrange("o (p g) -> (o p) g", p=128))
        w_slot = fin.tile([128, NG], bf16)
        nc.sync.dma_start(
            w_slot[:], wdram[0:1, :].rearrange("o (p g) -> (o p) g", p=128))

        # ---- focal epilogue on (128, 234)
        # p = exp(a)/S computed while the Exp table is still loaded, so ACT
        # swaps tables only once (Exp -> Ln).
        ea = fin.tile([128, NG], f32)
        nc.scalar.activation(ea[:], a_slot[:], Act.Exp)
        rS = fin.tile([128, NG], f32)
        nc.vector.reciprocal(rS[:], s_slot[:])
        pp = fin.tile([128, NG], f32)
        nc.vector.tensor_tensor(out=pp[:], in0=ea[:], in1=rS[:], op=Alu.mult)
        lnS = fin.tile([128, NG], f32)
        nc.scalar.activation(lnS[:], s_slot[:], Act.Ln)
        logp = fin.tile([128, NG], f32)
        nc.vector.tensor_tensor(out=logp[:], in0=a_slot[:], in1=lnS[:],
                                op=Alu.subtract)
        om = fin.tile([128, NG], f32)
        nc.vector.tensor_scalar(out=om[:], in0=pp[:], scalar1=-1.0,
                                scalar2=1.0, op0=Alu.mult, op1=Alu.add)
        om2 = fin.tile([128, NG], f32)
        nc.vector.tensor_tensor(out=om2[:], in0=om[:], in1=om[:], op=Alu.mult)
        t2 = fin.tile([128, NG], f32)
        nc.vector.scalar_tensor_tensor(
            out=t2[:], in0=om2[:], scalar=C0, in1=logp[:],
            op0=Alu.mult, op1=Alu.mult)
        fs = fin.tile([128, NG], f32)
        acc = fin.tile([128, 1], f32)
        nc.vector.scalar_tensor_tensor(
            out=fs[:], in0=t2[:], scalar=0.0, in1=w_slot[:],
            op0=Alu.add, op1=Alu.mult, accum_out=acc[:])
        nc.sync.dma_start(partial[:, :], acc[:])

    nc.compile()
    return nc


_CACHE = {}


def _get_program():
    if "nc" not in _CACHE:
        _CACHE["nc"] = build_program()
    return _CACHE["nc"]


def _bin_f32(d):
    """Exact f32 replication of the reference LID binning on box depths."""
    d = np.asarray(d, dtype=np.float32)
    idx = np.float32(-0.5) + np.float32(0.5) * np.sqrt(
        np.float32(1.0) + np.float32(8.0) * (d - np.float32(D_MIN))
        / np.float32(BIN_SIZE))
    invalid = (idx < 0) | (idx > NUM_BINS) | ~np.isfinite(idx)
    return np.where(invalid, NUM_BINS, idx.astype(np.int32)).astype(np.float32)


def kernel(depth_logits, gt_boxes2d, num_gt_per_img, gt_center_depth):
    global LAST_RESULTS
    dl = np.ascontiguousarray(np.asarray(depth_logits, dtype=np.float32))
    assert dl.shape == (B, C, H, W), dl.shape
    n_gt = int(num_gt_per_img)
    assert n_gt == N, n_gt
    boxes = np.asarray(gt_boxes2d, dtype=np.float32)
    depth = np.asarray(gt_center_depth, dtype=np.float32)

    u1 = np.floor(boxes[:, 0]).astype(np.int32)
    v1 = np.floor(boxes[:, 1]).astype(np.int32)
    u2 = np.ceil(boxes[:, 2]).astype(np.int32)
    v2 = np.ceil(boxes[:, 3]).astype(np.int32)
    bins = _bin_f32(depth)                                    # (B*N,)
    rows = np.arange(H)[:, None]
    cols = np.arange(W)[None, :]
    iota81 = np.arange(C, dtype=np.float32)[:, None]
    ones81 = np.ones((C, 1), dtype=_BF16)

    logits_flat = dl.reshape(B, C, HW)
    in_maps = []
    for b in range(B):
        sl = slice(b * N, (b + 1) * N)
        bv1, bv2, bu1, bu2 = v1[sl], v2[sl], u1[sl], u2[sl]
        bb = bins[sl]
        rp = np.where((rows >= bv1[None, :]) & (rows < bv2[None, :]),
                      0.0, BIGBIN).astype(np.float32)          # (H, N)
        cb = np.where((cols >= bu1[:, None]) & (cols < bu2[:, None]),
                      bb[:, None], BIGBIN).astype(_BF16)       # (N, W)
        in_maps.append({
            "logits": np.ascontiguousarray(logits_flat[b].astype(_BF16)),
            "rowpen": np.ascontiguousarray(rp),
            "colbin": np.ascontiguousarray(cb),
            "iota81": iota81,
            "ones81": ones81,
        })

    nc = _get_program()
    res = run_bass_kernel_spmd(nc, in_maps, core_ids=list(range(B)))
    LAST_RESULTS = res
    total = np.float64(0.0)
    for r in res.results:
        total += np.asarray(r["partial"], dtype=np.float64).sum()
    return np.float32(total)


if __name__ == "__main__":
    import tempfile
    from concourse.bass_utils import compile_bass_kernel
    compile_bass_kernel(_get_program(), tempfile.mkdtemp())
    print("COMPILE OK")
